# revision 27
# baseline (speedup 1.0000x reference)
"""AVAttention Trainium2 kernel.

Computes, for each sample b:
    k   = ph @ Wk + bk                 [S, D]
    q   = g.reshape(CF, T).T @ Wq + bq [T, D]
    att = softmax(mask(q @ k.T / sqrt(D)))   over S
    out = (att @ (ph @ Wv + bv)) @ Wmel + bmel  -> reshape [64, F, T]

Strategy: data-parallel over batch B=16 across 8 cores (2 samples/core).
Everything on-chip is kept in a "transposed" layout (feature dim on
partitions) so no attention-matrix transposes are ever needed:
  qT[d,t], kq[p,t], attT[s,t], exps[s,t], valueT[d,t], outT[m,t].
ph arrives host-pretransposed as phT[p,s].  The attention logits use
the associativity  att^T = phT^T @ (Wk^T @ qT)  so no k tensor is ever
materialized (bk cancels exactly in softmax: it adds a per-row
constant).  Softmax over s (the partition dim) uses a ones-column
matmul for the denominator and a K=1 ones-row matmul to broadcast
1/denom across partitions.  The length mask is folded into the exp()
activation as a per-partition bias (0 or -30000) precomputed host-side
from `lengths`.  bv is folded host-side into bmel (softmax weights sum
to 1), bq is applied per-partition during the qT copy.

Dtypes: the logit path (g, phT, Wq, Wk, Wmel, q, kq, valueT) runs in
float32r (TF32-like fp32: 1 PE cycle/row at N>=256, ~1e-4 rounding;
measured 272ns per 128x128x512 matmul vs 259ns for bf16).  The softmax
value path (exps, v) runs in bf16: softmax averaging shrinks those
rounding errors by ~sqrt(n).  All matmuls use N=512 so the 2-pass fp32
LDWEIGHTS (215ns) hides behind the matmul stream.  AVA_DT=bf16 runs
everything in bf16; AVA_DT=f32 runs the logit path in plain fp32.
"""

import math
import os

import numpy as np

import concourse.bacc as bacc
import concourse.mybir as mybir
import concourse.tile as tile
from concourse.bass_utils import run_bass_kernel_spmd

B, S, T = 16, 1024, 2048
CF = 2560          # q in_features = C*Fdim = 128*20
KO = CF // 128     # 20 contraction chunks for q projection
D = 512            # out_dim (k/q/v width); 4 partition blocks
PH = 512           # ph feature dim; 4 partition blocks
MEL = 1280         # out features; 10 partition blocks
N_CORES = 8
B_LOC = B // N_CORES
SCALE = 1.0 / math.sqrt(D)
MASK_NEG = -30000.0

_DT_NAME = os.environ.get("AVA_DT", "f32r")
DT = {
    "f32r": mybir.dt.float32r,
    "bf16": mybir.dt.bfloat16,
    "f32": mybir.dt.float32,
}[_DT_NAME]
# value-path dtype (exps, v): bf16 unless running the pure-f32 variant
DT2 = mybir.dt.float32 if _DT_NAME == "f32" else mybir.dt.bfloat16
TC = 512
NTC = T // TC
F32 = mybir.dt.float32
_BF = _DT_NAME == "bf16"


def _build_nc():
    nc = bacc.Bacc("TRN2", target_bir_lowering=False,
                   dynamic_dma_scratch_size=256)

    phT_d = nc.dram_tensor("phT", [B_LOC, 128, PH // 128, S], DT, kind="ExternalInput")
    g_d = nc.dram_tensor("g", [B_LOC, NTC, 128, KO, TC], DT, kind="ExternalInput")
    wq_d = nc.dram_tensor("wq", [128, KO, D], DT, kind="ExternalInput")
    wkT_d = nc.dram_tensor("wkT", [128, D // 128, PH], DT, kind="ExternalInput")
    wv_d = nc.dram_tensor("wv", [128, PH // 128, D], DT, kind="ExternalInput")
    wmel_d = nc.dram_tensor("wmel", [128, D // 128, MEL], DT, kind="ExternalInput")
    bq_d = nc.dram_tensor("bq", [128, D // 128], F32, kind="ExternalInput")
    bmel_d = nc.dram_tensor("bmel", [128, MEL // 128], F32, kind="ExternalInput")
    maskb_d = nc.dram_tensor("maskb", [128, B_LOC, S // 128], F32, kind="ExternalInput")
    ones_c_d = nc.dram_tensor("ones_c", [128, 1], DT2, kind="ExternalInput")
    ones_r_d = nc.dram_tensor("ones_r", [1, 128], DT, kind="ExternalInput")
    out_d = nc.dram_tensor("out", [B_LOC, 64, 20, T], F32, kind="ExternalOutput")

    with tile.TileContext(nc) as tc:
        with tc.tile_pool(name="const", bufs=1) as cpool, \
             tc.tile_pool(name="sb", bufs=2) as pool, \
             tc.tile_pool(name="ps", bufs=2, space="PSUM") as ps:

            # ---- constants / weights (resident) ----
            ones_col = cpool.tile([128, 1], DT2)
            nc.sync.dma_start(ones_col[:], ones_c_d[:])
            ones_row = cpool.tile([1, 128], DT)
            nc.sync.dma_start(ones_row[:], ones_r_d[:])

            wq_t = cpool.tile([128, KO, D], DT)
            nc.sync.dma_start(wq_t[:], wq_d[:])
            wkT_t = cpool.tile([128, D // 128, PH], DT)
            nc.sync.dma_start(wkT_t[:], wkT_d[:])
            wv_t = cpool.tile([128, PH // 128, D], DT)
            nc.sync.dma_start(wv_t[:], wv_d[:])
            wmel_t = cpool.tile([128, D // 128, MEL], DT)
            nc.sync.dma_start(wmel_t[:], wmel_d[:])
            bq_t = cpool.tile([128, D // 128], F32)
            nc.sync.dma_start(bq_t[:], bq_d[:])
            bmel_t = cpool.tile([128, MEL // 128], F32)
            nc.sync.dma_start(bmel_t[:], bmel_d[:])
            maskb_t = cpool.tile([128, B_LOC, S // 128], F32)
            nc.sync.dma_start(maskb_t[:], maskb_d[:])

            # ---------- software-pipelined schedule ----------
            # PE stream per unit u=(b,t), with prev = previous unit:
            #   qT g0(u) | bcast(prev) | qT g1-3(u) | valMM+TT(prev) |
            #   kq(u) [+ g/phT/v prefetch] | outT(prev) | attT(u)+den(u)
            # so every cross-engine drain is covered by unrelated PE work.
            units = [(b, t) for b in range(B_LOC) for t in range(NTC)]
            st = {}

            def emit_phT_dma(b):
                phT = pool.tile([128, PH // 128, S], DT, tag="phT", bufs=1,
                                name=f"phT_{b}")
                nc.scalar.dma_start(phT[:], phT_d[b])
                st[("phT", b)] = phT

            def emit_v(b):
                phT = st[("phT", b)]
                with nc.named_scope(f"v_{b}"):
                    v_sb = pool.tile([128, S // 128, D], DT2, tag="v", bufs=1,
                                     name=f"v_{b}")
                    for sb in range(S // 128):
                        pv = ps.tile([128, D], F32, tag="q", bufs=2,
                                     name=f"pv_{b}_{sb}")
                        for po in range(PH // 128):
                            nc.tensor.matmul(
                                pv[:],
                                phT[:, po, sb * 128:(sb + 1) * 128],
                                wv_t[:, po, :],
                                start=(po == 0), stop=(po == PH // 128 - 1),
                            )
                        nc.vector.tensor_copy(v_sb[:, sb, :], pv[:])
                st[("v", b)] = v_sb

            def emit_g_dma(u):
                b, t = u
                g_sb = pool.tile([128, KO, TC], DT, tag="g", bufs=2,
                                 name=f"g_{b}_{t}")
                for kq_ in range(4):
                    nc.scalar.dma_start(g_sb[:, 5 * kq_:5 * (kq_ + 1), :],
                                        g_d[b, t, :, 5 * kq_:5 * (kq_ + 1), :])
                st[("g", u)] = g_sb

            def emit_qT_group(u, db):
                b, t = u
                g_sb = st[("g", u)]
                if db == 0:
                    st[("qT", u)] = pool.tile([128, D // 128, TC], DT, tag="qT",
                                              bufs=1, name=f"qT_{b}_{t}")
                qT = st[("qT", u)]
                with nc.named_scope(f"qT_{b}_{t}_{db}"):
                    pq = ps.tile([128, TC], F32, tag="q", bufs=2,
                                 name=f"pq_{b}_{t}_{db}")
                    for ko in range(KO):
                        nc.tensor.matmul(
                            pq[:],
                            wq_t[:, ko, db * 128:(db + 1) * 128],
                            g_sb[:, ko, :],
                            start=(ko == 0), stop=(ko == KO - 1),
                        )
                    nc.vector.tensor_scalar_add(qT[:, db, :], pq[:],
                                                bq_t[:, db:db + 1])

            def emit_kq(u):
                b, t = u
                qT = st[("qT", u)]
                with nc.named_scope(f"kq_{b}_{t}"):
                    kq_sb = pool.tile([128, PH // 128, TC], DT, tag="kq",
                                      bufs=1, name=f"kq_{b}_{t}")
                    for pb_ in range(PH // 128):
                        pkq = ps.tile([128, TC], F32, tag="q", bufs=2,
                                      name=f"pkq_{b}_{t}_{pb_}")
                        for dc in range(D // 128):
                            nc.tensor.matmul(
                                pkq[:],
                                wkT_t[:, dc, pb_ * 128:(pb_ + 1) * 128],
                                qT[:, dc, :],
                                start=(dc == 0), stop=(dc == D // 128 - 1),
                            )
                        nc.scalar.copy(kq_sb[:, pb_, :], pkq[:])
                st[("kq", u)] = kq_sb

            def emit_attT_den(u):
                b, t = u
                kq_sb = st[("kq", u)]
                phT = st[("phT", b)]
                with nc.named_scope(f"att_{b}_{t}"):
                    exps = pool.tile([128, S // 128, TC], DT2, tag="exps",
                                     bufs=1, name=f"exps_{b}_{t}")
                    pd = ps.tile([1, TC], F32, tag="den", bufs=1,
                                 name=f"pd_{b}_{t}")
                    NSB = S // 128
                    pend = []
                    for sb in range(NSB):
                        pa = ps.tile([128, TC], F32, tag="att", bufs=2,
                                     name=f"pa_{b}_{t}_{sb}")
                        for po in range(PH // 128):
                            nc.tensor.matmul(
                                pa[:],
                                phT[:, po, sb * 128:(sb + 1) * 128],
                                kq_sb[:, po, :],
                                start=(po == 0), stop=(po == PH // 128 - 1),
                            )
                        nc.scalar.activation(
                            exps[:, sb, :], pa[:],
                            mybir.ActivationFunctionType.Exp,
                            bias=maskb_t[:, b, sb:sb + 1], scale=SCALE)
                        pend.append(sb)
                        # interleave denominator matmuls two groups behind
                        if sb >= 2:
                            dsb = pend.pop(0)
                            nc.tensor.matmul(pd[:], ones_col[:], exps[:, dsb, :],
                                             start=(dsb == 0), stop=False)
                    for dsb in pend:
                        nc.tensor.matmul(pd[:], ones_col[:], exps[:, dsb, :],
                                         start=False, stop=(dsb == NSB - 1))
                    den_rec_dt = pool.tile([1, TC], DT, tag="den_rec_dt", bufs=1,
                                           name=f"den_rec_dt_{b}_{t}")
                    if DT == F32:
                        nc.vector.reciprocal(den_rec_dt[:], pd[:])
                    else:
                        with nc.allow_low_precision(
                                reason="1/denominator in f32r (~1e-4) is fine"):
                            nc.vector.reciprocal(den_rec_dt[:], pd[:])
                st[("exps", u)] = exps
                st[("dd", u)] = den_rec_dt

            def emit_bcast(u):
                b, t = u
                with nc.named_scope(f"bc_{b}_{t}"):
                    pb = ps.tile([128, TC], F32, tag="den", bufs=1,
                                 name=f"pb_{b}_{t}")
                    nc.tensor.matmul(pb[:], ones_row[:], st[("dd", u)][:],
                                     start=True, stop=True)
                    recipb = pool.tile([128, TC], F32, tag="recipb", bufs=1,
                                       name=f"recipb_{b}_{t}")
                    nc.vector.tensor_copy(recipb[:], pb[:])
                st[("recipb", u)] = recipb

            def emit_val(u):
                b, t = u
                exps = st[("exps", u)]
                v_sb = st[("v", b)]
                recipb = st[("recipb", u)]
                with nc.named_scope(f"val_{b}_{t}"):
                    valT = pool.tile([128, D // 128, TC], DT, tag="valT",
                                     bufs=1, name=f"valT_{b}_{t}")
                    for db in range(D // 128):
                        pv2 = ps.tile([128, TC], F32, tag="vo", bufs=3,
                                      name=f"pv2_{b}_{t}_{db}")
                        for sb in range(S // 128):
                            nc.tensor.matmul(
                                pv2[:],
                                v_sb[:, sb, db * 128:(db + 1) * 128],
                                exps[:, sb, :],
                                start=(sb == 0), stop=(sb == S // 128 - 1),
                            )
                        nc.vector.tensor_tensor(valT[:, db, :], pv2[:],
                                                recipb[:], mybir.AluOpType.mult)
                st[("valT", u)] = valT

            def emit_out(u):
                b, t = u
                valT = st[("valT", u)]
                out_v = out_d[b].rearrange("h f t -> f h t")
                with nc.named_scope(f"out_{b}_{t}"):
                    for mb in range(MEL // 128):
                        po2 = ps.tile([128, TC], F32, tag="vo", bufs=3,
                                      name=f"po2_{b}_{t}_{mb}")
                        for db in range(D // 128):
                            nc.tensor.matmul(
                                po2[:],
                                wmel_t[:, db, mb * 128:(mb + 1) * 128],
                                valT[:, db, :],
                                start=(db == 0), stop=(db == D // 128 - 1),
                            )
                        out_sb = pool.tile([128, TC], F32, tag="out_sb", bufs=3,
                                           name=f"out_sb_{b}_{t}_{mb}")
                        if mb % 2 == 0:
                            nc.scalar.activation(
                                out_sb[:], po2[:],
                                mybir.ActivationFunctionType.Identity,
                                bias=bmel_t[:, mb:mb + 1], scale=1.0)
                        else:
                            nc.vector.tensor_scalar_add(out_sb[:], po2[:],
                                                        bmel_t[:, mb:mb + 1])
                        nc.sync.dma_start(
                            out_v[2 * mb:2 * mb + 2, :, t * TC:(t + 1) * TC],
                            out_sb[:])

            # ---------- pipeline driver ----------
            emit_phT_dma(0)
            emit_g_dma(units[0])
            emit_v(0)
            prev = None
            for idx, u in enumerate(units):
                emit_qT_group(u, 0)
                if prev is not None:
                    emit_bcast(prev)
                for db in range(1, D // 128):
                    emit_qT_group(u, db)
                if prev is not None:
                    emit_val(prev)
                emit_kq(u)
                if idx + 1 < len(units):
                    nxt = units[idx + 1]
                    if nxt[1] == 0:
                        emit_phT_dma(nxt[0])
                    emit_g_dma(nxt)
                    if nxt[1] == 0:
                        emit_v(nxt[0])
                if prev is not None:
                    emit_out(prev)
                emit_attT_den(u)
                prev = u
            emit_bcast(prev)
            emit_val(prev)
            emit_out(prev)

    nc.compile()
    return nc


def _np_dt(x):
    x = np.asarray(x, dtype=np.float32)
    if _BF:
        import ml_dtypes
        return np.ascontiguousarray(x.astype(ml_dtypes.bfloat16))
    return np.ascontiguousarray(x)


def _np_dt2(x):
    x = np.asarray(x, dtype=np.float32)
    if _DT_NAME == "f32":
        return np.ascontiguousarray(x)
    import ml_dtypes
    return np.ascontiguousarray(x.astype(ml_dtypes.bfloat16))


def kernel(ph, g, lengths, Wk, bk, Wv, bv, Wq, bq, Wmel, bmel, **_):
    ph = np.asarray(ph, dtype=np.float32)
    g = np.asarray(g, dtype=np.float32)
    lengths = np.asarray(lengths)
    Wk = np.asarray(Wk, dtype=np.float32)
    Wv = np.asarray(Wv, dtype=np.float32)
    bv = np.asarray(bv, dtype=np.float32)
    Wq = np.asarray(Wq, dtype=np.float32)
    bq = np.asarray(bq, dtype=np.float32)
    Wmel = np.asarray(Wmel, dtype=np.float32)
    bmel = np.asarray(bmel, dtype=np.float32)

    # host-side prearrangement into device layouts
    g_h = g.reshape(B, KO, 128, NTC, TC).transpose(0, 3, 2, 1, 4)
    g_h = _np_dt(g_h)                              # [B, NTC, 128, KO, TC]
    phT_h = _np_dt(ph.transpose(0, 2, 1)           # [B, PH, S]
                   .reshape(B, PH // 128, 128, S)
                   .transpose(0, 2, 1, 3))         # [B, 128, PH//128, S]
    wq_h = _np_dt(Wq.reshape(KO, 128, D).transpose(1, 0, 2))
    wkT_h = _np_dt(Wk.T.reshape(D // 128, 128, PH).transpose(1, 0, 2))
    wv_h = _np_dt(Wv.reshape(PH // 128, 128, D).transpose(1, 0, 2))
    wmel_h = _np_dt(Wmel.reshape(D // 128, 128, MEL).transpose(1, 0, 2))
    bq_h = np.ascontiguousarray(bq.reshape(D // 128, 128).T)
    bmel_eff = (bv.astype(np.float64) @ Wmel.astype(np.float64)
                + bmel.astype(np.float64)).astype(np.float32)
    bmel_h = np.ascontiguousarray(bmel_eff.reshape(MEL // 128, 128).T)

    sidx = np.arange(S).reshape(S // 128, 128).T   # [128, S//128] (p, sb)
    valid = sidx[:, None, :] < lengths.astype(np.int64)[None, :, None]
    maskb = np.where(valid, 0.0, MASK_NEG).astype(np.float32)  # [128, B, S//128]

    ones_c_h = _np_dt2(np.ones((128, 1), np.float32))
    ones_r_h = _np_dt(np.ones((1, 128), np.float32))

    nc = _build_nc()

    in_maps = []
    for c in range(N_CORES):
        sl = slice(c * B_LOC, (c + 1) * B_LOC)
        in_maps.append({
            "phT": np.ascontiguousarray(phT_h[sl]),
            "g": np.ascontiguousarray(g_h[sl]),
            "wq": wq_h, "wkT": wkT_h, "wv": wv_h, "wmel": wmel_h,
            "bq": bq_h, "bmel": bmel_h,
            "maskb": np.ascontiguousarray(maskb[:, sl, :]),
            "ones_c": ones_c_h, "ones_r": ones_r_h,
        })

    res = run_bass_kernel_spmd(nc, in_maps, core_ids=list(range(N_CORES)))
    out = np.concatenate([res.results[c]["out"] for c in range(N_CORES)], axis=0)
    return out
